# revision 5
# baseline (speedup 1.0000x reference)
"""MoE (token-choice top-2 router + grouped SwiGLU experts + shared expert)
on 8 Trainium2 NeuronCores.

Sharding: expert-parallel — core e owns expert e's routed tokens (host
dispatch, capacity-padded), plus a 1/8 data-parallel slice of the shared
expert. Host does the routing control plane (gate matmul, top-2, stable
sort, gather/scale, final scatter-add combine); the device does all the
FLOPs in bf16 with fp32 PSUM accumulation.

v2 redesign (trace-driven, from the 397us baseline):
- ALL operands are SBUF-resident before use. Weights arrive via
  host-side swizzled DRAM layouts so every DMA moves 11-12KB contiguous
  per-partition lines (the v1 baseline streamed weight tiles as 256B
  descriptors, which capped the weight stream at ~60-80GB/s and starved
  the PE at kernel start and each phase transition).
- Weight tensors are chunked along the output (m) dim into 4 chunks,
  loaded through an 8-slot rotating tile pool: at any time one phase's
  full weight tensor + the next tensor's prefetch are in flight. Slot
  recycling gives the prefetch pipeline for free via tile deps.
- Custom per-phase matmul loops (no composable_matmul_tile_kernel):
  r1 is n-outer so the first x n-chunk + first w1 m-chunk unblock the
  PE ~8us in; r3/out_r are n-inner so consecutive matmuls share the
  stationary weight tile. PSUM pool of 6 banks keeps deep pipelining.
- Outputs staged in SBUF (bf16) and written as 2.9-4KB-line DMAs in a
  swizzled DRAM layout (host unswizzles); final flush is one DMA.

Self-contained: only needs numpy/ml_dtypes/concourse (the Bass stack).
"""

import math
import os

import numpy as np
import ml_dtypes

BF16 = ml_dtypes.bfloat16
NCORES = 8
TOP_K = 2
ROUTE_SCALE = 1.0
P = 128

# filled by the last kernel() call (exec_time_ns etc. when tracing)
LAST = {}

_PROGRAM_CACHE = {}


def _install_profhook():
    """Best-effort shim for antenv.axon_hooks so trace=True can capture NTFF
    profiles in this container. Harmless no-op if anything is missing."""
    try:
        import sys
        import types

        if "antenv.axon_hooks" in sys.modules:
            return
        import trn_agent_boot.trn_boot as tb

        hook = tb._ntff_profile_via_ctypes("/opt/axon/libaxon_pjrt.so")
        m = types.ModuleType("antenv.axon_hooks")
        m._hook = hook
        m.set_axon_ntff_profile_hook = lambda h: setattr(m, "_hook", h)
        m.get_axon_ntff_profile_hook = lambda: m._hook
        import antenv

        sys.modules["antenv.axon_hooks"] = m
        antenv.axon_hooks = m

        import concourse.bass_utils as bu

        bu.upload_artifacts = lambda tmpdir: tmpdir
    except Exception:
        pass


def _free_div(n):
    """Largest f = n/k (k<=4) with f <= 512, preferring big f."""
    for k in (1, 2, 3, 4):
        if n % k == 0 and n // k <= 512:
            return n // k
    for f in (512, 384, 256, 128):
        if n % f == 0:
            return f
    raise ValueError(f"no free-dim divisor for {n}")


def _pick_ntok(nmax, cap):
    """Smallest n in [nmax, cap] whose free-dim divides nicely (PSUM <=512)."""
    for n in range(nmax, cap + 1):
        try:
            _free_div(n)
            return n
        except ValueError:
            continue
    return cap


def _mchunks(n_mtiles, n_chunks):
    """Split n_mtiles 128-col m-tiles into n_chunks contiguous chunks."""
    base = n_mtiles // n_chunks
    rem = n_mtiles % n_chunks
    sizes = [base + (1 if i < rem else 0) for i in range(n_chunks)]
    out = []
    s = 0
    for sz in sizes:
        out.append((s, sz))
        s += sz
    return out


WCH = 4  # m-chunks per weight tensor


def _build_program(D, H, NTOK, TS):
    import concourse.bacc as bacc
    import concourse.tile as tile
    from concourse import mybir
    from contextlib import ExitStack

    bf = mybir.dt.bfloat16
    f32 = mybir.dt.float32

    KD = D // P  # 16 k-subtiles for the D-contraction (up-proj)
    KH = H // P  # 11 k-subtiles for the H-contraction (out-proj)
    MT_H = H // P  # 11 m-tiles over H
    MT_D = D // P  # 16 m-tiles over D
    FREE_R = _free_div(NTOK)
    NCH_R = NTOK // FREE_R
    FREE_S = _free_div(TS)
    NCH_S = TS // FREE_S
    XKC = 2  # k-chunks for the xr prefetch (first-tile latency)
    assert KD % XKC == 0
    KDC = KD // XKC

    up_chunks = _mchunks(MT_H, WCH)  # chunks of H m-tiles (w1/w3/sw1/sw3)
    dn_chunks = _mchunks(MT_D, WCH)  # chunks of D m-tiles (w2/sw2)

    nc = bacc.Bacc(target_bir_lowering=False)

    # --- DRAM tensors (all host-swizzled; per-partition-contiguous lines) ---
    # up-weight chunk c: [P, KD*csz*P] row-major; (p, ks, j) = wT[ks*P+p, c0*P+j]
    def wdecl(name, nk, chunks):
        return [
            nc.dram_tensor(f"{name}{c}", [P, nk * sz * P], bf, kind="ExternalInput")
            for c, (_, sz) in enumerate(chunks)
        ]

    w1d = wdecl("w1", KD, up_chunks)
    w3d = wdecl("w3", KD, up_chunks)
    sw1d = wdecl("sw1", KD, up_chunks)
    sw3d = wdecl("sw3", KD, up_chunks)
    w2d = wdecl("w2", KH, dn_chunks)
    sw2d = wdecl("sw2", KH, dn_chunks)
    # xr chunk (n, kc): [P, KDC*FREE_R]; (p, s, j) = xrT[(kc*KDC+s)*P+p, n*FREE_R+j]
    xrd = [
        [
            nc.dram_tensor(f"xr{n}_{kc}", [P, KDC * FREE_R], bf, kind="ExternalInput")
            for kc in range(XKC)
        ]
        for n in range(NCH_R)
    ]
    xsd = nc.dram_tensor("xs", [P, KD * TS], bf, kind="ExternalInput")
    # outputs (swizzled, host unswizzles): routed [WCH, NCH_R, P, msz*FREE_R]
    outr = nc.dram_tensor(
        "outr", [WCH, NCH_R, P, max(sz for _, sz in dn_chunks) * FREE_R], bf,
        kind="ExternalOutput",
    )
    outs = nc.dram_tensor(
        "outs", [WCH, NCH_S, P, max(sz for _, sz in dn_chunks) * FREE_S], bf,
        kind="ExternalOutput",
    )

    with tile.TileContext(nc) as tc, ExitStack() as ctx:
        caches = ctx.enter_context(tc.tile_pool(name="caches", bufs=1))
        # persistent activation caches
        xr_t = [
            [
                caches.tile(
                    [P, KDC, FREE_R], bf, tag=f"xr{n}_{kc}", name=f"xr{n}_{kc}"
                )
                for kc in range(XKC)
            ]
            for n in range(NCH_R)
        ]
        xs_t = caches.tile([P, KD, TS], bf, tag="xs")
        h1c = caches.tile([P, MT_H, NTOK], bf, tag="h1c")
        h1s = caches.tile([P, MT_H, TS], bf, tag="h1s")

        wpool = ctx.enter_context(tc.tile_pool(name="wpool", bufs=7))
        psum = ctx.enter_context(tc.tile_pool(name="psum", bufs=6, space="PSUM"))
        stgp = ctx.enter_context(tc.tile_pool(name="stg", bufs=1))

        # ---- prefetch issues (engine FIFO order = pacing) ----
        # scalar queue: xr chunks then xs
        for n in range(NCH_R):
            for kc in range(XKC):
                nc.scalar.dma_start(
                    out=xr_t[n][kc][:],
                    in_=xrd[n][kc][:].rearrange("p (s j) -> p s j", s=KDC),
                )
        nc.scalar.dma_start(
            out=xs_t[:], in_=xsd[:].rearrange("p (s j) -> p s j", s=KD)
        )

        # sync queue: weight chunks in consumption order; the 8-slot pool
        # rotation makes later tensors' DMAs wait for the earlier tensors'
        # readers automatically (prefetch pipeline).
        def wload(dram_chunks, nk, chunks, label):
            tiles = []
            for c, (_, sz) in enumerate(chunks):
                t = wpool.tile([P, nk, sz * P], bf, tag="w", name=f"{label}{c}")
                nc.sync.dma_start(
                    out=t[:],
                    in_=dram_chunks[c][:].rearrange("p (s j) -> p s j", s=nk),
                )
                tiles.append(t)
            return tiles

        w1t = wload(w1d, KD, up_chunks, "w1t")
        w3t = wload(w3d, KD, up_chunks, "w3t")
        sw1t = wload(sw1d, KD, up_chunks, "sw1t")
        sw3t = wload(sw3d, KD, up_chunks, "sw3t")
        w2t = wload(w2d, KH, dn_chunks, "w2t")
        sw2t = wload(sw2d, KH, dn_chunks, "sw2t")

        Silu = mybir.ActivationFunctionType.Silu

        def xr_rhs(n, ks):
            return xr_t[n][ks // KDC][:, ks % KDC, :]

        # ---- phase r1: h1 = silu(w1T.T @ xr), n-outer (stream-friendly) ----
        for n in range(NCH_R):
            for c, (m0, msz) in enumerate(up_chunks):
                for mi in range(msz):
                    ps = psum.tile([P, 512], f32, tag="ps", name=f"ps_r1_{n}_{c}_{mi}")
                    for ks in range(KD):
                        nc.tensor.matmul(
                            ps[:, :FREE_R],
                            w1t[c][:, ks, mi * P : (mi + 1) * P],
                            xr_rhs(n, ks),
                            start=(ks == 0),
                            stop=(ks == KD - 1),
                        )
                    nc.scalar.activation(
                        h1c[:, m0 + mi, n * FREE_R : (n + 1) * FREE_R],
                        ps[:, :FREE_R],
                        Silu,
                    )

        # ---- phase r3: h1 *= (w3T.T @ xr), n-inner (weight reuse) ----
        for c, (m0, msz) in enumerate(up_chunks):
            for mi in range(msz):
                pss = [
                    psum.tile([P, 512], f32, tag="ps", name=f"ps_r3_{c}_{mi}_{n}")
                    for n in range(NCH_R)
                ]
                for ks in range(KD):
                    for n in range(NCH_R):
                        nc.tensor.matmul(
                            pss[n][:, :FREE_R],
                            w3t[c][:, ks, mi * P : (mi + 1) * P],
                            xr_rhs(n, ks),
                            start=(ks == 0),
                            stop=(ks == KD - 1),
                        )
                for n in range(NCH_R):
                    sl = h1c[:, m0 + mi, n * FREE_R : (n + 1) * FREE_R]
                    nc.vector.tensor_mul(out=sl, in0=pss[n][:, :FREE_R], in1=sl)

        # ---- phase s1/s3: shared-expert swiglu on xs ----
        for wt, is_mul in ((sw1t, False), (sw3t, True)):
            for c, (m0, msz) in enumerate(up_chunks):
                for mi in range(msz):
                    pss = [
                        psum.tile([P, 512], f32, tag="ps", name=f"ps_s_{c}_{mi}_{n}")
                        for n in range(NCH_S)
                    ]
                    for ks in range(KD):
                        for n in range(NCH_S):
                            nc.tensor.matmul(
                                pss[n][:, :FREE_S],
                                wt[c][:, ks, mi * P : (mi + 1) * P],
                                xs_t[:, ks, n * FREE_S : (n + 1) * FREE_S],
                                start=(ks == 0),
                                stop=(ks == KD - 1),
                            )
                    for n in range(NCH_S):
                        sl = h1s[:, m0 + mi, n * FREE_S : (n + 1) * FREE_S]
                        if is_mul:
                            nc.vector.tensor_mul(
                                out=sl, in0=pss[n][:, :FREE_S], in1=sl
                            )
                        else:
                            nc.scalar.activation(sl, pss[n][:, :FREE_S], Silu)

        # ---- phase out_r: outrT = w2T.T @ h1 (n-inner; vector copies,
        # scalar DMAs) ----
        for c, (m0, msz) in enumerate(dn_chunks):
            stgs = [
                stgp.tile([P, msz, FREE_R], bf, tag="stgr", bufs=4, name=f"stgr{c}_{n}")
                for n in range(NCH_R)
            ]
            for mi in range(msz):
                pss = [
                    psum.tile([P, 512], f32, tag="ps", name=f"ps_or_{c}_{mi}_{n}")
                    for n in range(NCH_R)
                ]
                for ks in range(KH):
                    for n in range(NCH_R):
                        nc.tensor.matmul(
                            pss[n][:, :FREE_R],
                            w2t[c][:, ks, mi * P : (mi + 1) * P],
                            h1c[:, ks, n * FREE_R : (n + 1) * FREE_R],
                            start=(ks == 0),
                            stop=(ks == KH - 1),
                        )
                for n in range(NCH_R):
                    nc.vector.tensor_copy(
                        out=stgs[n][:, mi, :], in_=pss[n][:, :FREE_R]
                    )
            for n in range(NCH_R):
                nc.scalar.dma_start(
                    out=outr[c, n, :, : msz * FREE_R].rearrange(
                        "p (s j) -> p s j", s=msz
                    ),
                    in_=stgs[n][:],
                )

        # ---- phase out_s: outsT = sw2T.T @ h1s (scalar copies, sync DMAs) ----
        for c, (m0, msz) in enumerate(dn_chunks):
            stgs = [
                stgp.tile([P, msz, FREE_S], bf, tag="stgs", bufs=2, name=f"stgs{c}_{n}")
                for n in range(NCH_S)
            ]
            for mi in range(msz):
                pss = [
                    psum.tile([P, 512], f32, tag="ps", name=f"ps_os_{c}_{mi}_{n}")
                    for n in range(NCH_S)
                ]
                for ks in range(KH):
                    for n in range(NCH_S):
                        nc.tensor.matmul(
                            pss[n][:, :FREE_S],
                            sw2t[c][:, ks, mi * P : (mi + 1) * P],
                            h1s[:, ks, n * FREE_S : (n + 1) * FREE_S],
                            start=(ks == 0),
                            stop=(ks == KH - 1),
                        )
                for n in range(NCH_S):
                    nc.scalar.activation(
                        stgs[n][:, mi, :],
                        pss[n][:, :FREE_S],
                        mybir.ActivationFunctionType.Copy,
                    )
            for n in range(NCH_S):
                nc.sync.dma_start(
                    out=outs[c, n, :, : msz * FREE_S].rearrange(
                        "p (s j) -> p s j", s=msz
                    ),
                    in_=stgs[n][:],
                )

    nc.compile()
    return nc


def _route(x, gate_w, expert_bias):
    """Host control plane mirroring the reference routing exactly."""
    BS, SLEN, D = x.shape
    T = BS * SLEN
    xt = np.ascontiguousarray(x.reshape(T, D), dtype=np.float32)
    logits = xt @ gate_w.astype(np.float32).T  # [T, E]
    scores = 1.0 / (1.0 + np.exp(-logits))
    biased = scores + np.asarray(expert_bias, np.float32)[None, :]
    sel = np.argsort(-biased, axis=1, kind="stable")[:, :TOP_K]  # [T, K]
    top_scores = np.take_along_axis(scores, sel, axis=1) * ROUTE_SCALE
    sel_flat = sel.reshape(-1)
    order = np.argsort(sel_flat, kind="stable")  # [T*K]
    counts = np.bincount(sel_flat, minlength=NCORES)
    tok_idx = order // TOP_K
    scores_sorted = top_scores.reshape(-1)[order].astype(np.float32)
    return xt, counts, tok_idx, scores_sorted


def _swz_w(wT, nk, chunks):
    """wT [K, M] f32 -> list of [P, nk*sz*P] bf16 swizzled chunks."""
    K, M = wT.shape
    w3d = wT.reshape(nk, P, M)  # (ks, p, m)
    out = []
    for m0, sz in chunks:
        blk = w3d[:, :, m0 * P : (m0 + sz) * P]  # (ks, p, c)
        out.append(
            np.ascontiguousarray(blk.transpose(1, 0, 2)).reshape(P, nk * sz * P)
            .astype(BF16)
        )
    return out


def kernel(x, gate_w, w1, w2, w3, sw1, sw2, sw3, expert_bias):
    from concourse.bass_utils import run_bass_kernel_spmd

    x = np.asarray(x, np.float32)
    gate_w = np.asarray(gate_w, np.float32)
    w1 = np.asarray(w1, np.float32)
    w2 = np.asarray(w2, np.float32)
    w3 = np.asarray(w3, np.float32)
    sw1 = np.asarray(sw1, np.float32)
    sw2 = np.asarray(sw2, np.float32)
    sw3 = np.asarray(sw3, np.float32)
    expert_bias = np.asarray(expert_bias, np.float32)
    BS, SLEN, D = x.shape
    T = BS * SLEN
    H = w1.shape[1]
    TS = T // NCORES
    KD = D // P
    KH = H // P
    MT_H = H // P
    MT_D = D // P

    xt, counts, tok_idx, scores_sorted = _route(x, gate_w, expert_bias)
    off = np.concatenate([[0], np.cumsum(counts)]).astype(np.int64)
    CAP = max(128, int(math.ceil(counts.max() / 128) * 128))
    NTOK = _pick_ntok(max(128, int(counts.max())), CAP)
    FREE_R = _free_div(NTOK)
    NCH_R = NTOK // FREE_R
    FREE_S = _free_div(TS)
    NCH_S = TS // FREE_S
    XKC = 2
    KDC = KD // XKC
    up_chunks = _mchunks(MT_H, WCH)
    dn_chunks = _mchunks(MT_D, WCH)

    key = (D, H, NTOK, TS)
    if key not in _PROGRAM_CACHE:
        _PROGRAM_CACHE[key] = _build_program(D, H, NTOK, TS)
    nc = _PROGRAM_CACHE[key]

    # ---- stage per-core inputs (swizzled) ----
    shared_chunks = {}
    for name, wmat, nk, chunks in (
        ("sw1", sw1.T, KD, up_chunks),
        ("sw3", sw3.T, KD, up_chunks),
        ("sw2", sw2.T, KH, dn_chunks),
    ):
        for c, arr in enumerate(_swz_w(np.ascontiguousarray(wmat), nk, chunks)):
            shared_chunks[f"{name}{c}"] = arr

    in_maps = []
    for e in range(NCORES):
        n_e = int(counts[e])
        idx = tok_idx[off[e] : off[e] + n_e]
        seg = xt[idx] * scores_sorted[off[e] : off[e] + n_e, None]  # [n_e, D]
        xrT = np.zeros((D, NTOK), np.float32)
        xrT[:, :n_e] = seg.T
        xr4 = xrT.reshape(XKC, KDC, P, NTOK)  # (kc, s, p, tok)
        im = dict(shared_chunks)
        for n in range(NCH_R):
            for kc in range(XKC):
                blk = xr4[kc, :, :, n * FREE_R : (n + 1) * FREE_R]
                im[f"xr{n}_{kc}"] = (
                    np.ascontiguousarray(blk.transpose(1, 0, 2))
                    .reshape(P, KDC * FREE_R)
                    .astype(BF16)
                )
        xsT = np.ascontiguousarray(xt[e * TS : (e + 1) * TS].T)  # [D, TS]
        im["xs"] = (
            xsT.reshape(KD, P, TS).transpose(1, 0, 2).reshape(P, KD * TS).astype(BF16)
        )
        for name, wmat, nk, chunks in (
            ("w1", w1[e].T, KD, up_chunks),
            ("w3", w3[e].T, KD, up_chunks),
            ("w2", w2[e].T, KH, dn_chunks),
        ):
            for c, arr in enumerate(
                _swz_w(np.ascontiguousarray(wmat), nk, chunks)
            ):
                im[f"{name}{c}"] = arr
        in_maps.append(im)

    trace = os.environ.get("KERNEL_TRACE", "") not in ("", "0")
    if trace:
        _install_profhook()
    res = run_bass_kernel_spmd(nc, in_maps, list(range(NCORES)), trace=trace)
    LAST["exec_time_ns"] = res.exec_time_ns
    LAST["results"] = res

    # ---- combine: unswizzle outputs, shared slices + routed scatter-add ----
    mszs = [sz for _, sz in dn_chunks]
    msz_max = max(mszs)

    def unswz(arr, nch, free):
        # arr [WCH, nch, P, msz_max*free] -> [D, nch*free]
        full = np.empty((MT_D * P, nch * free), np.float32)
        for c, (m0, sz) in enumerate(dn_chunks):
            blk = arr[c, :, :, : sz * free].reshape(nch, P, sz, free)
            # (n, p, s, j) -> rows (m0+s)*P+p, cols n*free+j
            blk = blk.transpose(2, 1, 0, 3).reshape(sz * P, nch * free)
            full[m0 * P : (m0 + sz) * P] = blk
        return full

    out = np.empty((T, D), np.float32)
    for c in range(NCORES):
        osw = np.asarray(res.results[c]["outs"], dtype=np.float32)
        out[c * TS : (c + 1) * TS] = unswz(osw, NCH_S, FREE_S).T
    for e in range(NCORES):
        n_e = int(counts[e])
        if n_e:
            orw = np.asarray(res.results[e]["outr"], dtype=np.float32)
            full = unswz(orw, NCH_R, FREE_R)  # [D, NTOK]
            idx = tok_idx[off[e] : off[e] + n_e]
            out[idx] += full[:, :n_e].T
    return out.reshape(BS, SLEN, D)


# revision 8
# speedup vs baseline: 1.1945x; 1.1945x over previous
"""MoE (token-choice top-2 router + grouped SwiGLU experts + shared expert)
on 8 Trainium2 NeuronCores.

Sharding: expert-parallel — core e owns expert e's routed tokens (host
dispatch, capacity-padded), plus a 1/8 data-parallel slice of the shared
expert. Host does the routing control plane (gate matmul, top-2, stable
sort, gather/scale, final scatter-add combine); the device does all the
FLOPs in bf16 with fp32 PSUM accumulation.

v2 redesign (trace-driven, from the 397us baseline):
- ALL operands are SBUF-resident before use. Weights arrive via
  host-side swizzled DRAM layouts so every DMA moves 11-12KB contiguous
  per-partition lines (the v1 baseline streamed weight tiles as 256B
  descriptors, which capped the weight stream at ~60-80GB/s and starved
  the PE at kernel start and each phase transition).
- Weight tensors are chunked along the output (m) dim into 4 chunks,
  loaded through an 8-slot rotating tile pool: at any time one phase's
  full weight tensor + the next tensor's prefetch are in flight. Slot
  recycling gives the prefetch pipeline for free via tile deps.
- Custom per-phase matmul loops (no composable_matmul_tile_kernel):
  r1 is n-outer so the first x n-chunk + first w1 m-chunk unblock the
  PE ~8us in; r3/out_r are n-inner so consecutive matmuls share the
  stationary weight tile. PSUM pool of 6 banks keeps deep pipelining.
- Outputs staged in SBUF (bf16) and written as 2.9-4KB-line DMAs in a
  swizzled DRAM layout (host unswizzles); final flush is one DMA.

Self-contained: only needs numpy/ml_dtypes/concourse (the Bass stack).
"""

import math
import os

import numpy as np
import ml_dtypes

BF16 = ml_dtypes.bfloat16
NCORES = 8
TOP_K = 2
ROUTE_SCALE = 1.0
P = 128

# filled by the last kernel() call (exec_time_ns etc. when tracing)
LAST = {}

_PROGRAM_CACHE = {}


def _install_profhook():
    """Best-effort shim for antenv.axon_hooks so trace=True can capture NTFF
    profiles in this container. Harmless no-op if anything is missing."""
    try:
        import sys
        import types

        if "antenv.axon_hooks" in sys.modules:
            return
        import trn_agent_boot.trn_boot as tb

        hook = tb._ntff_profile_via_ctypes("/opt/axon/libaxon_pjrt.so")
        m = types.ModuleType("antenv.axon_hooks")
        m._hook = hook
        m.set_axon_ntff_profile_hook = lambda h: setattr(m, "_hook", h)
        m.get_axon_ntff_profile_hook = lambda: m._hook
        import antenv

        sys.modules["antenv.axon_hooks"] = m
        antenv.axon_hooks = m

        import concourse.bass_utils as bu

        bu.upload_artifacts = lambda tmpdir: tmpdir
    except Exception:
        pass


def _free_div(n):
    """Largest f = n/k (k<=4) with f <= 512, preferring big f."""
    for k in (1, 2, 3, 4):
        if n % k == 0 and n // k <= 512:
            return n // k
    for f in (512, 384, 256, 128):
        if n % f == 0:
            return f
    raise ValueError(f"no free-dim divisor for {n}")


def _pick_ntok(nmax, cap):
    """Smallest n in [nmax, cap] whose free-dim divides nicely (PSUM <=512)."""
    for n in range(nmax, cap + 1):
        try:
            _free_div(n)
            return n
        except ValueError:
            continue
    return cap


def _mchunks(n_mtiles, n_chunks):
    """Split n_mtiles 128-col m-tiles into n_chunks contiguous chunks."""
    base = n_mtiles // n_chunks
    rem = n_mtiles % n_chunks
    sizes = [base + (1 if i < rem else 0) for i in range(n_chunks)]
    out = []
    s = 0
    for sz in sizes:
        out.append((s, sz))
        s += sz
    return out


WCH = 4  # m-chunks per weight tensor


def _build_program(D, H, NTOK, TS):
    import concourse.bacc as bacc
    import concourse.tile as tile
    from concourse import mybir
    from contextlib import ExitStack

    bf = mybir.dt.bfloat16
    f32 = mybir.dt.float32

    KD = D // P  # 16 k-subtiles for the D-contraction (up-proj)
    KH = H // P  # 11 k-subtiles for the H-contraction (out-proj)
    MT_H = H // P  # 11 m-tiles over H
    MT_D = D // P  # 16 m-tiles over D
    FREE_R = _free_div(NTOK)
    NCH_R = NTOK // FREE_R
    FREE_S = _free_div(TS)
    NCH_S = TS // FREE_S
    XKC = 2  # k-chunks for the xr prefetch (first-tile latency)
    assert KD % XKC == 0
    KDC = KD // XKC

    up_chunks = _mchunks(MT_H, WCH)  # chunks of H m-tiles (w1/w3/sw1/sw3)
    dn_chunks = _mchunks(MT_D, WCH)  # chunks of D m-tiles (w2/sw2)

    nc = bacc.Bacc(target_bir_lowering=False)

    # --- DRAM tensors (all host-swizzled; per-partition-contiguous lines) ---
    # up-weight chunk c: [P, KD*csz*P] row-major; (p, ks, j) = wT[ks*P+p, c0*P+j]
    def wdecl(name, nk, chunks):
        return [
            nc.dram_tensor(f"{name}{c}", [P, nk * sz * P], bf, kind="ExternalInput")
            for c, (_, sz) in enumerate(chunks)
        ]

    w1d = wdecl("w1", KD, up_chunks)
    w3d = wdecl("w3", KD, up_chunks)
    sw1d = wdecl("sw1", KD, up_chunks)
    sw3d = wdecl("sw3", KD, up_chunks)
    w2d = wdecl("w2", KH, dn_chunks)
    sw2d = wdecl("sw2", KH, dn_chunks)
    # xr chunk (n, kc): [P, KDC*FREE_R]; (p, s, j) = xrT[(kc*KDC+s)*P+p, n*FREE_R+j]
    xrd = [
        [
            nc.dram_tensor(f"xr{n}_{kc}", [P, KDC * FREE_R], bf, kind="ExternalInput")
            for kc in range(XKC)
        ]
        for n in range(NCH_R)
    ]
    xsd = nc.dram_tensor("xs", [P, KD * TS], bf, kind="ExternalInput")
    # outputs (swizzled, host unswizzles): routed [WCH, NCH_R, P, msz*FREE_R]
    outr = nc.dram_tensor(
        "outr", [WCH, NCH_R, P, max(sz for _, sz in dn_chunks) * FREE_R], bf,
        kind="ExternalOutput",
    )
    outs = nc.dram_tensor(
        "outs", [WCH, NCH_S, P, max(sz for _, sz in dn_chunks) * FREE_S], bf,
        kind="ExternalOutput",
    )

    with tile.TileContext(nc) as tc, ExitStack() as ctx:
        caches = ctx.enter_context(tc.tile_pool(name="caches", bufs=1))
        # persistent activation caches
        xr_t = [
            [
                caches.tile(
                    [P, KDC, FREE_R], bf, tag=f"xr{n}_{kc}", name=f"xr{n}_{kc}"
                )
                for kc in range(XKC)
            ]
            for n in range(NCH_R)
        ]
        xs_t = caches.tile([P, KD, TS], bf, tag="xs")
        h1c = caches.tile([P, MT_H, NTOK], bf, tag="h1c")
        h1s = caches.tile([P, MT_H, TS], bf, tag="h1s")

        wpool = ctx.enter_context(tc.tile_pool(name="wpool", bufs=7))
        psum = ctx.enter_context(tc.tile_pool(name="psum", bufs=6, space="PSUM"))
        stgp = ctx.enter_context(tc.tile_pool(name="stg", bufs=1))

        # ---- prefetch issues (engine FIFO order = pacing) ----
        # scalar queue: xr chunks then xs
        for n in range(NCH_R):
            for kc in range(XKC):
                nc.scalar.dma_start(
                    out=xr_t[n][kc][:],
                    in_=xrd[n][kc][:].rearrange("p (s j) -> p s j", s=KDC),
                )
        nc.scalar.dma_start(
            out=xs_t[:], in_=xsd[:].rearrange("p (s j) -> p s j", s=KD)
        )

        # sync queue: weight chunks in consumption order; the 8-slot pool
        # rotation makes later tensors' DMAs wait for the earlier tensors'
        # readers automatically (prefetch pipeline).
        def wload(dram_chunks, nk, chunks, label):
            # chunk layout [P, mi, ks, 128]: the k-sweep for a fixed m-tile
            # walks contiguous 256B blocks (strided LDWEIGHTS reads measure
            # ~20% slower PE streaming).
            tiles = []
            for c, (_, sz) in enumerate(chunks):
                t = wpool.tile([P, sz, nk, P], bf, tag="w", name=f"{label}{c}")
                nc.sync.dma_start(
                    out=t[:],
                    in_=dram_chunks[c][:].rearrange(
                        "p (m s j) -> p m s j", m=sz, s=nk
                    ),
                )
                tiles.append(t)
            return tiles

        w1t = wload(w1d, KD, up_chunks, "w1t")
        w3t = wload(w3d, KD, up_chunks, "w3t")
        sw1t = wload(sw1d, KD, up_chunks, "sw1t")
        sw3t = wload(sw3d, KD, up_chunks, "sw3t")
        w2t = wload(w2d, KH, dn_chunks, "w2t")
        sw2t = wload(sw2d, KH, dn_chunks, "sw2t")

        Silu = mybir.ActivationFunctionType.Silu

        def xr_rhs(n, ks):
            return xr_t[n][ks // KDC][:, ks % KDC, :]

        # ---- phase r1: h1 = silu(w1T.T @ xr), n-outer (stream-friendly) ----
        for n in range(NCH_R):
            for c, (m0, msz) in enumerate(up_chunks):
                for mi in range(msz):
                    ps = psum.tile([P, 512], f32, tag="ps", name=f"ps_r1_{n}_{c}_{mi}")
                    for ks in range(KD):
                        nc.tensor.matmul(
                            ps[:, :FREE_R],
                            w1t[c][:, mi, ks, :],
                            xr_rhs(n, ks),
                            start=(ks == 0),
                            stop=(ks == KD - 1),
                        )
                    nc.scalar.activation(
                        h1c[:, m0 + mi, n * FREE_R : (n + 1) * FREE_R],
                        ps[:, :FREE_R],
                        Silu,
                    )

        # ---- phase r3: h1 *= (w3T.T @ xr), n-inner (weight reuse) ----
        for c, (m0, msz) in enumerate(up_chunks):
            for mi in range(msz):
                pss = [
                    psum.tile([P, 512], f32, tag="ps", name=f"ps_r3_{c}_{mi}_{n}")
                    for n in range(NCH_R)
                ]
                for ks in range(KD):
                    for n in range(NCH_R):
                        nc.tensor.matmul(
                            pss[n][:, :FREE_R],
                            w3t[c][:, mi, ks, :],
                            xr_rhs(n, ks),
                            start=(ks == 0),
                            stop=(ks == KD - 1),
                        )
                for n in range(NCH_R):
                    sl = h1c[:, m0 + mi, n * FREE_R : (n + 1) * FREE_R]
                    nc.vector.tensor_mul(out=sl, in0=pss[n][:, :FREE_R], in1=sl)

        # ---- phase s1/s3: shared-expert swiglu on xs ----
        for wt, is_mul in ((sw1t, False), (sw3t, True)):
            for c, (m0, msz) in enumerate(up_chunks):
                for mi in range(msz):
                    pss = [
                        psum.tile([P, 512], f32, tag="ps", name=f"ps_s_{c}_{mi}_{n}")
                        for n in range(NCH_S)
                    ]
                    for ks in range(KD):
                        for n in range(NCH_S):
                            nc.tensor.matmul(
                                pss[n][:, :FREE_S],
                                wt[c][:, mi, ks, :],
                                xs_t[:, ks, n * FREE_S : (n + 1) * FREE_S],
                                start=(ks == 0),
                                stop=(ks == KD - 1),
                            )
                    for n in range(NCH_S):
                        sl = h1s[:, m0 + mi, n * FREE_S : (n + 1) * FREE_S]
                        if is_mul:
                            nc.vector.tensor_mul(
                                out=sl, in0=pss[n][:, :FREE_S], in1=sl
                            )
                        else:
                            nc.scalar.activation(sl, pss[n][:, :FREE_S], Silu)

        # ---- phase out_r: outrT = w2T.T @ h1 (n-inner; vector copies,
        # scalar DMAs) ----
        for c, (m0, msz) in enumerate(dn_chunks):
            stgs = [
                stgp.tile([P, msz, FREE_R], bf, tag="stgr", bufs=4, name=f"stgr{c}_{n}")
                for n in range(NCH_R)
            ]
            for mi in range(msz):
                pss = [
                    psum.tile([P, 512], f32, tag="ps", name=f"ps_or_{c}_{mi}_{n}")
                    for n in range(NCH_R)
                ]
                for ks in range(KH):
                    for n in range(NCH_R):
                        nc.tensor.matmul(
                            pss[n][:, :FREE_R],
                            w2t[c][:, mi, ks, :],
                            h1c[:, ks, n * FREE_R : (n + 1) * FREE_R],
                            start=(ks == 0),
                            stop=(ks == KH - 1),
                        )
                for n in range(NCH_R):
                    nc.vector.tensor_copy(
                        out=stgs[n][:, mi, :], in_=pss[n][:, :FREE_R]
                    )
            for n in range(NCH_R):
                nc.scalar.dma_start(
                    out=outr[c, n, :, : msz * FREE_R].rearrange(
                        "p (s j) -> p s j", s=msz
                    ),
                    in_=stgs[n][:],
                )

        # ---- phase out_s: outsT = sw2T.T @ h1s (scalar copies, sync DMAs) ----
        for c, (m0, msz) in enumerate(dn_chunks):
            stgs = [
                stgp.tile([P, msz, FREE_S], bf, tag="stgs", bufs=2, name=f"stgs{c}_{n}")
                for n in range(NCH_S)
            ]
            for mi in range(msz):
                pss = [
                    psum.tile([P, 512], f32, tag="ps", name=f"ps_os_{c}_{mi}_{n}")
                    for n in range(NCH_S)
                ]
                for ks in range(KH):
                    for n in range(NCH_S):
                        nc.tensor.matmul(
                            pss[n][:, :FREE_S],
                            sw2t[c][:, mi, ks, :],
                            h1s[:, ks, n * FREE_S : (n + 1) * FREE_S],
                            start=(ks == 0),
                            stop=(ks == KH - 1),
                        )
                for n in range(NCH_S):
                    nc.scalar.activation(
                        stgs[n][:, mi, :],
                        pss[n][:, :FREE_S],
                        mybir.ActivationFunctionType.Copy,
                    )
            for n in range(NCH_S):
                nc.sync.dma_start(
                    out=outs[c, n, :, : msz * FREE_S].rearrange(
                        "p (s j) -> p s j", s=msz
                    ),
                    in_=stgs[n][:],
                )

    nc.compile()
    return nc


def _route(x, gate_w, expert_bias):
    """Host control plane mirroring the reference routing exactly."""
    BS, SLEN, D = x.shape
    T = BS * SLEN
    xt = np.ascontiguousarray(x.reshape(T, D), dtype=np.float32)
    logits = xt @ gate_w.astype(np.float32).T  # [T, E]
    scores = 1.0 / (1.0 + np.exp(-logits))
    biased = scores + np.asarray(expert_bias, np.float32)[None, :]
    sel = np.argsort(-biased, axis=1, kind="stable")[:, :TOP_K]  # [T, K]
    top_scores = np.take_along_axis(scores, sel, axis=1) * ROUTE_SCALE
    sel_flat = sel.reshape(-1)
    order = np.argsort(sel_flat, kind="stable")  # [T*K]
    counts = np.bincount(sel_flat, minlength=NCORES)
    tok_idx = order // TOP_K
    scores_sorted = top_scores.reshape(-1)[order].astype(np.float32)
    return xt, counts, tok_idx, scores_sorted


def _swz_w(wT, nk, chunks):
    """wT [K, M] f32 -> list of [P, sz*nk*P] bf16 swizzled chunks with
    per-partition layout (mi, ks, j): the device k-sweep for a fixed m-tile
    reads contiguous 256B blocks."""
    K, M = wT.shape
    w3d = wT.reshape(nk, P, M)  # (ks, p, m)
    out = []
    for m0, sz in chunks:
        blk = w3d[:, :, m0 * P : (m0 + sz) * P].reshape(nk, P, sz, P)
        # (ks, p, mi, j) -> (p, mi, ks, j)
        out.append(
            np.ascontiguousarray(blk.transpose(1, 2, 0, 3))
            .reshape(P, sz * nk * P)
            .astype(BF16)
        )
    return out


def kernel(x, gate_w, w1, w2, w3, sw1, sw2, sw3, expert_bias):
    from concourse.bass_utils import run_bass_kernel_spmd

    x = np.asarray(x, np.float32)
    gate_w = np.asarray(gate_w, np.float32)
    w1 = np.asarray(w1, np.float32)
    w2 = np.asarray(w2, np.float32)
    w3 = np.asarray(w3, np.float32)
    sw1 = np.asarray(sw1, np.float32)
    sw2 = np.asarray(sw2, np.float32)
    sw3 = np.asarray(sw3, np.float32)
    expert_bias = np.asarray(expert_bias, np.float32)
    BS, SLEN, D = x.shape
    T = BS * SLEN
    H = w1.shape[1]
    TS = T // NCORES
    KD = D // P
    KH = H // P
    MT_H = H // P
    MT_D = D // P

    xt, counts, tok_idx, scores_sorted = _route(x, gate_w, expert_bias)
    off = np.concatenate([[0], np.cumsum(counts)]).astype(np.int64)
    CAP = max(128, int(math.ceil(counts.max() / 128) * 128))
    NTOK = _pick_ntok(max(128, int(counts.max())), CAP)
    FREE_R = _free_div(NTOK)
    NCH_R = NTOK // FREE_R
    FREE_S = _free_div(TS)
    NCH_S = TS // FREE_S
    XKC = 2
    KDC = KD // XKC
    up_chunks = _mchunks(MT_H, WCH)
    dn_chunks = _mchunks(MT_D, WCH)

    key = (D, H, NTOK, TS)
    if key not in _PROGRAM_CACHE:
        _PROGRAM_CACHE[key] = _build_program(D, H, NTOK, TS)
    nc = _PROGRAM_CACHE[key]

    # ---- stage per-core inputs (swizzled) ----
    shared_chunks = {}
    for name, wmat, nk, chunks in (
        ("sw1", sw1.T, KD, up_chunks),
        ("sw3", sw3.T, KD, up_chunks),
        ("sw2", sw2.T, KH, dn_chunks),
    ):
        for c, arr in enumerate(_swz_w(np.ascontiguousarray(wmat), nk, chunks)):
            shared_chunks[f"{name}{c}"] = arr

    in_maps = []
    for e in range(NCORES):
        n_e = int(counts[e])
        idx = tok_idx[off[e] : off[e] + n_e]
        seg = xt[idx] * scores_sorted[off[e] : off[e] + n_e, None]  # [n_e, D]
        xrT = np.zeros((D, NTOK), np.float32)
        xrT[:, :n_e] = seg.T
        xr4 = xrT.reshape(XKC, KDC, P, NTOK)  # (kc, s, p, tok)
        im = dict(shared_chunks)
        for n in range(NCH_R):
            for kc in range(XKC):
                blk = xr4[kc, :, :, n * FREE_R : (n + 1) * FREE_R]
                im[f"xr{n}_{kc}"] = (
                    np.ascontiguousarray(blk.transpose(1, 0, 2))
                    .reshape(P, KDC * FREE_R)
                    .astype(BF16)
                )
        xsT = np.ascontiguousarray(xt[e * TS : (e + 1) * TS].T)  # [D, TS]
        im["xs"] = (
            xsT.reshape(KD, P, TS).transpose(1, 0, 2).reshape(P, KD * TS).astype(BF16)
        )
        for name, wmat, nk, chunks in (
            ("w1", w1[e].T, KD, up_chunks),
            ("w3", w3[e].T, KD, up_chunks),
            ("w2", w2[e].T, KH, dn_chunks),
        ):
            for c, arr in enumerate(
                _swz_w(np.ascontiguousarray(wmat), nk, chunks)
            ):
                im[f"{name}{c}"] = arr
        in_maps.append(im)

    trace = os.environ.get("KERNEL_TRACE", "") not in ("", "0")
    if trace:
        _install_profhook()
    res = run_bass_kernel_spmd(nc, in_maps, list(range(NCORES)), trace=trace)
    LAST["exec_time_ns"] = res.exec_time_ns
    LAST["results"] = res

    # ---- combine: unswizzle outputs, shared slices + routed scatter-add ----
    mszs = [sz for _, sz in dn_chunks]
    msz_max = max(mszs)

    def unswz(arr, nch, free):
        # arr [WCH, nch, P, msz_max*free] -> [D, nch*free]
        full = np.empty((MT_D * P, nch * free), np.float32)
        for c, (m0, sz) in enumerate(dn_chunks):
            blk = arr[c, :, :, : sz * free].reshape(nch, P, sz, free)
            # (n, p, s, j) -> rows (m0+s)*P+p, cols n*free+j
            blk = blk.transpose(2, 1, 0, 3).reshape(sz * P, nch * free)
            full[m0 * P : (m0 + sz) * P] = blk
        return full

    out = np.empty((T, D), np.float32)
    for c in range(NCORES):
        osw = np.asarray(res.results[c]["outs"], dtype=np.float32)
        out[c * TS : (c + 1) * TS] = unswz(osw, NCH_S, FREE_S).T
    for e in range(NCORES):
        n_e = int(counts[e])
        if n_e:
            orw = np.asarray(res.results[e]["outr"], dtype=np.float32)
            full = unswz(orw, NCH_R, FREE_R)  # [D, NTOK]
            idx = tok_idx[off[e] : off[e] + n_e]
            out[idx] += full[:, :n_e].T
    return out.reshape(BS, SLEN, D)


# revision 18
# speedup vs baseline: 1.2063x; 1.0099x over previous
"""MoE (token-choice top-2 router + grouped SwiGLU experts + shared expert)
on 8 Trainium2 NeuronCores.

Sharding: expert-parallel — core e owns expert e's routed tokens (host
dispatch, capacity-padded), plus a 1/8 data-parallel slice of the shared
expert. Host does the routing control plane (gate matmul, top-2, stable
sort, gather/scale, final scatter-add combine); the device does all the
FLOPs in bf16 with fp32 PSUM accumulation.

v2 redesign (trace-driven, from the 397us baseline):
- ALL operands are SBUF-resident before use. Weights arrive via
  host-side swizzled DRAM layouts so every DMA moves 11-12KB contiguous
  per-partition lines (the v1 baseline streamed weight tiles as 256B
  descriptors, which capped the weight stream at ~60-80GB/s and starved
  the PE at kernel start and each phase transition).
- Weight tensors are chunked along the output (m) dim into 4 chunks,
  loaded through an 8-slot rotating tile pool: at any time one phase's
  full weight tensor + the next tensor's prefetch are in flight. Slot
  recycling gives the prefetch pipeline for free via tile deps.
- Custom per-phase matmul loops (no composable_matmul_tile_kernel):
  r1 is n-outer so the first x n-chunk + first w1 m-chunk unblock the
  PE ~8us in; r3/out_r are n-inner so consecutive matmuls share the
  stationary weight tile. PSUM pool of 6 banks keeps deep pipelining.
- Outputs staged in SBUF (bf16) and written as 2.9-4KB-line DMAs in a
  swizzled DRAM layout (host unswizzles); final flush is one DMA.

Self-contained: only needs numpy/ml_dtypes/concourse (the Bass stack).
"""

import math
import os

import numpy as np
import ml_dtypes

BF16 = ml_dtypes.bfloat16
NCORES = 8
TOP_K = 2
ROUTE_SCALE = 1.0
P = 128

# filled by the last kernel() call (exec_time_ns etc. when tracing)
LAST = {}

_PROGRAM_CACHE = {}


def _install_profhook():
    """Best-effort shim for antenv.axon_hooks so trace=True can capture NTFF
    profiles in this container. Harmless no-op if anything is missing."""
    try:
        import sys
        import types

        if "antenv.axon_hooks" in sys.modules:
            return
        import trn_agent_boot.trn_boot as tb

        hook = tb._ntff_profile_via_ctypes("/opt/axon/libaxon_pjrt.so")
        m = types.ModuleType("antenv.axon_hooks")
        m._hook = hook
        m.set_axon_ntff_profile_hook = lambda h: setattr(m, "_hook", h)
        m.get_axon_ntff_profile_hook = lambda: m._hook
        import antenv

        sys.modules["antenv.axon_hooks"] = m
        antenv.axon_hooks = m

        import concourse.bass_utils as bu

        bu.upload_artifacts = lambda tmpdir: tmpdir
    except Exception:
        pass


def _free_div(n):
    """Largest f = n/k (k<=4) with f <= 512, preferring big f."""
    for k in (1, 2, 3, 4):
        if n % k == 0 and n // k <= 512:
            return n // k
    for f in (512, 384, 256, 128):
        if n % f == 0:
            return f
    raise ValueError(f"no free-dim divisor for {n}")


def _pick_ntok(nmax, cap):
    """Smallest n in [nmax, cap] whose free-dim divides nicely (PSUM <=512)."""
    for n in range(nmax, cap + 1):
        try:
            _free_div(n)
            return n
        except ValueError:
            continue
    return cap


def _mchunks(n_mtiles, n_chunks):
    """Split n_mtiles 128-col m-tiles into n_chunks contiguous chunks,
    smallest chunk FIRST (the first chunk gates the kernel head)."""
    base = n_mtiles // n_chunks
    rem = n_mtiles % n_chunks
    sizes = [base] * (n_chunks - rem) + [base + 1] * rem
    out = []
    s = 0
    for sz in sizes:
        out.append((s, sz))
        s += sz
    return out


WCH = 4  # m-chunks per weight tensor


def _build_program(D, H, NTOK, TS):
    import concourse.bacc as bacc
    import concourse.tile as tile
    from concourse import mybir
    from contextlib import ExitStack

    bf = mybir.dt.bfloat16
    f32 = mybir.dt.float32

    KD = D // P  # 16 k-subtiles for the D-contraction (up-proj)
    KH = H // P  # 11 k-subtiles for the H-contraction (out-proj)
    MT_H = H // P  # 11 m-tiles over H
    MT_D = D // P  # 16 m-tiles over D
    FREE_R = _free_div(NTOK)
    NCH_R = NTOK // FREE_R
    FREE_S = _free_div(TS)
    NCH_S = TS // FREE_S
    XKC = 2  # k-chunks for the xr prefetch (first-tile latency)
    assert KD % XKC == 0
    KDC = KD // XKC

    up_chunks = _mchunks(MT_H, WCH)  # chunks of H m-tiles (w1/w3/sw1/sw3)
    dn_chunks = _mchunks(MT_D, WCH)  # chunks of D m-tiles (w2/sw2)

    nc = bacc.Bacc(target_bir_lowering=False)

    # --- DRAM tensors (all host-swizzled; per-partition-contiguous lines) ---
    # up-weight chunk c: [P, KD*csz*P] row-major; (p, ks, j) = wT[ks*P+p, c0*P+j]
    def wdecl(name, nk, chunks):
        return [
            nc.dram_tensor(f"{name}{c}", [P, nk * sz * P], bf, kind="ExternalInput")
            for c, (_, sz) in enumerate(chunks)
        ]

    w1d = wdecl("w1", KD, up_chunks)
    w3d = wdecl("w3", KD, up_chunks)
    sw1d = wdecl("sw1", KD, up_chunks)
    sw3d = wdecl("sw3", KD, up_chunks)
    w2d = wdecl("w2", KH, dn_chunks)
    sw2d = wdecl("sw2", KH, dn_chunks)
    # xr chunk (n, kc): [P, KDC*FREE_R]; (p, s, j) = xrT[(kc*KDC+s)*P+p, n*FREE_R+j]
    xrd = [
        [
            nc.dram_tensor(f"xr{n}_{kc}", [P, KDC * FREE_R], bf, kind="ExternalInput")
            for kc in range(XKC)
        ]
        for n in range(NCH_R)
    ]
    xsd = nc.dram_tensor("xs", [P, KD * TS], bf, kind="ExternalInput")
    # outputs (swizzled, host unswizzles): routed [WCH, NCH_R, P, msz*FREE_R]
    outr = nc.dram_tensor(
        "outr", [WCH, NCH_R, P, max(sz for _, sz in dn_chunks) * FREE_R], bf,
        kind="ExternalOutput",
    )
    outs = nc.dram_tensor(
        "outs", [WCH, NCH_S, P, max(sz for _, sz in dn_chunks) * FREE_S], bf,
        kind="ExternalOutput",
    )

    with tile.TileContext(nc) as tc, ExitStack() as ctx:
        caches = ctx.enter_context(tc.tile_pool(name="caches", bufs=1))
        # persistent activation caches
        xr_t = [
            [
                caches.tile(
                    [P, KDC, FREE_R], bf, tag=f"xr{n}_{kc}", name=f"xr{n}_{kc}"
                )
                for kc in range(XKC)
            ]
            for n in range(NCH_R)
        ]
        xs_t = caches.tile([P, KD, TS], bf, tag="xs")
        h1c = caches.tile([P, MT_H, NTOK], bf, tag="h1c")
        h1s = caches.tile([P, MT_H, TS], bf, tag="h1s")

        wpool = ctx.enter_context(tc.tile_pool(name="wpool", bufs=7))
        psum = ctx.enter_context(tc.tile_pool(name="psum", bufs=6, space="PSUM"))
        stgp = ctx.enter_context(tc.tile_pool(name="stg", bufs=1))

        # ---- prefetch issues (engine FIFO order = pacing) ----
        # scalar queue: xr n-chunks then xs
        for n in range(NCH_R):
            for kc in range(XKC):
                nc.scalar.dma_start(
                    out=xr_t[n][kc][:],
                    in_=xrd[n][kc][:].rearrange("p (s j) -> p s j", s=KDC),
                )
        nc.scalar.dma_start(
            out=xs_t[:], in_=xsd[:].rearrange("p (s j) -> p s j", s=KD)
        )

        # sync queue: weight chunks in consumption order; the 8-slot pool
        # rotation makes later tensors' DMAs wait for the earlier tensors'
        # readers automatically (prefetch pipeline).
        def wload(dram_chunks, nk, chunks, label, split0=False):
            # chunk layout [P, mi, ks, 128]: the k-sweep for a fixed m-tile
            # walks contiguous 256B blocks (strided LDWEIGHTS reads measure
            # ~20% slower PE streaming). split0: load chunk 0 with per-m-tile
            # DMAs so the first matmul only gates on one m-tile (subtile deps).
            tiles = []
            for c, (_, sz) in enumerate(chunks):
                t = wpool.tile([P, sz, nk, P], bf, tag="w", name=f"{label}{c}")
                src = dram_chunks[c][:].rearrange("p (m s j) -> p m s j", m=sz, s=nk)
                if split0 and c == 0:
                    for mi in range(sz):
                        nc.sync.dma_start(
                            out=t[:, mi : mi + 1], in_=src[:, mi : mi + 1]
                        )
                else:
                    nc.sync.dma_start(out=t[:], in_=src)
                tiles.append(t)
            return tiles

        w1t = wload(w1d, KD, up_chunks, "w1t", split0=True)
        w3t = wload(w3d, KD, up_chunks, "w3t")
        sw1t = wload(sw1d, KD, up_chunks, "sw1t")
        sw3t = wload(sw3d, KD, up_chunks, "sw3t")
        w2t = wload(w2d, KH, dn_chunks, "w2t")
        sw2t = wload(sw2d, KH, dn_chunks, "sw2t")

        Silu = mybir.ActivationFunctionType.Silu

        def xr_rhs(n, ks):
            return xr_t[n][ks // KDC][:, ks % KDC, :]

        # ---- phase r1: h1 = silu(w1T.T @ xr), n-outer (stream-friendly) ----
        for n in range(NCH_R):
            for c, (m0, msz) in enumerate(up_chunks):
                for mi in range(msz):
                    ps = psum.tile([P, 512], f32, tag="ps", name=f"ps_r1_{n}_{c}_{mi}")
                    for ks in range(KD):
                        nc.tensor.matmul(
                            ps[:, :FREE_R],
                            w1t[c][:, mi, ks, :],
                            xr_rhs(n, ks),
                            start=(ks == 0),
                            stop=(ks == KD - 1),
                        )
                    nc.scalar.activation(
                        h1c[:, m0 + mi, n * FREE_R : (n + 1) * FREE_R],
                        ps[:, :FREE_R],
                        Silu,
                    )

        # ---- phase r3: h1 *= (w3T.T @ xr), n-inner (weight reuse) ----
        for c, (m0, msz) in enumerate(up_chunks):
            for mi in range(msz):
                pss = [
                    psum.tile([P, 512], f32, tag="ps", name=f"ps_r3_{c}_{mi}_{n}")
                    for n in range(NCH_R)
                ]
                for ks in range(KD):
                    for n in range(NCH_R):
                        nc.tensor.matmul(
                            pss[n][:, :FREE_R],
                            w3t[c][:, mi, ks, :],
                            xr_rhs(n, ks),
                            start=(ks == 0),
                            stop=(ks == KD - 1),
                        )
                for n in range(NCH_R):
                    sl = h1c[:, m0 + mi, n * FREE_R : (n + 1) * FREE_R]
                    nc.vector.tensor_mul(out=sl, in0=pss[n][:, :FREE_R], in1=sl)

        # ---- phase s1/s3: shared-expert swiglu on xs ----
        for wt, is_mul in ((sw1t, False), (sw3t, True)):
            for c, (m0, msz) in enumerate(up_chunks):
                for mi in range(msz):
                    pss = [
                        psum.tile([P, 512], f32, tag="ps", name=f"ps_s_{c}_{mi}_{n}")
                        for n in range(NCH_S)
                    ]
                    for ks in range(KD):
                        for n in range(NCH_S):
                            nc.tensor.matmul(
                                pss[n][:, :FREE_S],
                                wt[c][:, mi, ks, :],
                                xs_t[:, ks, n * FREE_S : (n + 1) * FREE_S],
                                start=(ks == 0),
                                stop=(ks == KD - 1),
                            )
                    for n in range(NCH_S):
                        sl = h1s[:, m0 + mi, n * FREE_S : (n + 1) * FREE_S]
                        if is_mul:
                            nc.vector.tensor_mul(
                                out=sl, in0=pss[n][:, :FREE_S], in1=sl
                            )
                        else:
                            nc.scalar.activation(sl, pss[n][:, :FREE_S], Silu)

        # ---- phase out_r: outrT = w2T.T @ h1 (n-inner; vector copies,
        # scalar DMAs) ----
        for c, (m0, msz) in enumerate(dn_chunks):
            stgs = [
                stgp.tile([P, msz, FREE_R], bf, tag="stgr", bufs=4, name=f"stgr{c}_{n}")
                for n in range(NCH_R)
            ]
            for mi in range(msz):
                pss = [
                    psum.tile([P, 512], f32, tag="ps", name=f"ps_or_{c}_{mi}_{n}")
                    for n in range(NCH_R)
                ]
                for ks in range(KH):
                    for n in range(NCH_R):
                        nc.tensor.matmul(
                            pss[n][:, :FREE_R],
                            w2t[c][:, mi, ks, :],
                            h1c[:, ks, n * FREE_R : (n + 1) * FREE_R],
                            start=(ks == 0),
                            stop=(ks == KH - 1),
                        )
                for n in range(NCH_R):
                    nc.vector.tensor_copy(
                        out=stgs[n][:, mi, :], in_=pss[n][:, :FREE_R]
                    )
            for n in range(NCH_R):
                nc.scalar.dma_start(
                    out=outr[c, n, :, : msz * FREE_R].rearrange(
                        "p (s j) -> p s j", s=msz
                    ),
                    in_=stgs[n][:],
                )

        # ---- phase out_s: outsT = sw2T.T @ h1s (scalar copies, sync DMAs) ----
        for c, (m0, msz) in enumerate(dn_chunks):
            stgs = [
                stgp.tile([P, msz, FREE_S], bf, tag="stgs", bufs=2, name=f"stgs{c}_{n}")
                for n in range(NCH_S)
            ]
            for mi in range(msz):
                pss = [
                    psum.tile([P, 512], f32, tag="ps", name=f"ps_os_{c}_{mi}_{n}")
                    for n in range(NCH_S)
                ]
                for ks in range(KH):
                    for n in range(NCH_S):
                        nc.tensor.matmul(
                            pss[n][:, :FREE_S],
                            sw2t[c][:, mi, ks, :],
                            h1s[:, ks, n * FREE_S : (n + 1) * FREE_S],
                            start=(ks == 0),
                            stop=(ks == KH - 1),
                        )
                for n in range(NCH_S):
                    nc.scalar.activation(
                        stgs[n][:, mi, :],
                        pss[n][:, :FREE_S],
                        mybir.ActivationFunctionType.Copy,
                    )
            for n in range(NCH_S):
                nc.sync.dma_start(
                    out=outs[c, n, :, : msz * FREE_S].rearrange(
                        "p (s j) -> p s j", s=msz
                    ),
                    in_=stgs[n][:],
                )

    nc.compile()
    return nc


def _route(x, gate_w, expert_bias):
    """Host control plane mirroring the reference routing exactly."""
    BS, SLEN, D = x.shape
    T = BS * SLEN
    xt = np.ascontiguousarray(x.reshape(T, D), dtype=np.float32)
    logits = xt @ gate_w.astype(np.float32).T  # [T, E]
    scores = 1.0 / (1.0 + np.exp(-logits))
    biased = scores + np.asarray(expert_bias, np.float32)[None, :]
    sel = np.argsort(-biased, axis=1, kind="stable")[:, :TOP_K]  # [T, K]
    top_scores = np.take_along_axis(scores, sel, axis=1) * ROUTE_SCALE
    sel_flat = sel.reshape(-1)
    order = np.argsort(sel_flat, kind="stable")  # [T*K]
    counts = np.bincount(sel_flat, minlength=NCORES)
    tok_idx = order // TOP_K
    scores_sorted = top_scores.reshape(-1)[order].astype(np.float32)
    return xt, counts, tok_idx, scores_sorted


def _swz_w(wT, nk, chunks):
    """wT [K, M] f32 -> list of [P, sz*nk*P] bf16 swizzled chunks with
    per-partition layout (mi, ks, j): the device k-sweep for a fixed m-tile
    reads contiguous 256B blocks."""
    K, M = wT.shape
    w3d = wT.reshape(nk, P, M)  # (ks, p, m)
    out = []
    for m0, sz in chunks:
        blk = w3d[:, :, m0 * P : (m0 + sz) * P].reshape(nk, P, sz, P)
        # (ks, p, mi, j) -> (p, mi, ks, j)
        out.append(
            np.ascontiguousarray(blk.transpose(1, 2, 0, 3))
            .reshape(P, sz * nk * P)
            .astype(BF16)
        )
    return out


def kernel(x, gate_w, w1, w2, w3, sw1, sw2, sw3, expert_bias):
    from concourse.bass_utils import run_bass_kernel_spmd

    x = np.asarray(x, np.float32)
    gate_w = np.asarray(gate_w, np.float32)
    w1 = np.asarray(w1, np.float32)
    w2 = np.asarray(w2, np.float32)
    w3 = np.asarray(w3, np.float32)
    sw1 = np.asarray(sw1, np.float32)
    sw2 = np.asarray(sw2, np.float32)
    sw3 = np.asarray(sw3, np.float32)
    expert_bias = np.asarray(expert_bias, np.float32)
    BS, SLEN, D = x.shape
    T = BS * SLEN
    H = w1.shape[1]
    TS = T // NCORES
    KD = D // P
    KH = H // P
    MT_H = H // P
    MT_D = D // P

    xt, counts, tok_idx, scores_sorted = _route(x, gate_w, expert_bias)
    off = np.concatenate([[0], np.cumsum(counts)]).astype(np.int64)
    CAP = max(128, int(math.ceil(counts.max() / 128) * 128))
    NTOK = _pick_ntok(max(128, int(counts.max())), CAP)
    FREE_R = _free_div(NTOK)
    NCH_R = NTOK // FREE_R
    FREE_S = _free_div(TS)
    NCH_S = TS // FREE_S
    XKC = 2
    KDC = KD // XKC
    up_chunks = _mchunks(MT_H, WCH)
    dn_chunks = _mchunks(MT_D, WCH)

    key = (D, H, NTOK, TS)
    if key not in _PROGRAM_CACHE:
        _PROGRAM_CACHE[key] = _build_program(D, H, NTOK, TS)
    nc = _PROGRAM_CACHE[key]

    # ---- stage per-core inputs (swizzled) ----
    shared_chunks = {}
    for name, wmat, nk, chunks in (
        ("sw1", sw1.T, KD, up_chunks),
        ("sw3", sw3.T, KD, up_chunks),
        ("sw2", sw2.T, KH, dn_chunks),
    ):
        for c, arr in enumerate(_swz_w(np.ascontiguousarray(wmat), nk, chunks)):
            shared_chunks[f"{name}{c}"] = arr

    in_maps = []
    for e in range(NCORES):
        n_e = int(counts[e])
        idx = tok_idx[off[e] : off[e] + n_e]
        seg = xt[idx] * scores_sorted[off[e] : off[e] + n_e, None]  # [n_e, D]
        xrT = np.zeros((D, NTOK), np.float32)
        xrT[:, :n_e] = seg.T
        xr4 = xrT.reshape(XKC, KDC, P, NTOK)  # (kc, s, p, tok)
        im = dict(shared_chunks)
        for n in range(NCH_R):
            for kc in range(XKC):
                blk = xr4[kc, :, :, n * FREE_R : (n + 1) * FREE_R]
                im[f"xr{n}_{kc}"] = (
                    np.ascontiguousarray(blk.transpose(1, 0, 2))
                    .reshape(P, KDC * FREE_R)
                    .astype(BF16)
                )
        xsT = np.ascontiguousarray(xt[e * TS : (e + 1) * TS].T)  # [D, TS]
        im["xs"] = (
            xsT.reshape(KD, P, TS).transpose(1, 0, 2).reshape(P, KD * TS).astype(BF16)
        )
        for name, wmat, nk, chunks in (
            ("w1", w1[e].T, KD, up_chunks),
            ("w3", w3[e].T, KD, up_chunks),
            ("w2", w2[e].T, KH, dn_chunks),
        ):
            for c, arr in enumerate(
                _swz_w(np.ascontiguousarray(wmat), nk, chunks)
            ):
                im[f"{name}{c}"] = arr
        in_maps.append(im)

    trace = os.environ.get("KERNEL_TRACE", "") not in ("", "0")
    if trace:
        _install_profhook()
    res = run_bass_kernel_spmd(nc, in_maps, list(range(NCORES)), trace=trace)
    LAST["exec_time_ns"] = res.exec_time_ns
    LAST["results"] = res

    # ---- combine: unswizzle outputs, shared slices + routed scatter-add ----
    mszs = [sz for _, sz in dn_chunks]
    msz_max = max(mszs)

    def unswz(arr, nch, free):
        # arr [WCH, nch, P, msz_max*free] -> [D, nch*free]
        full = np.empty((MT_D * P, nch * free), np.float32)
        for c, (m0, sz) in enumerate(dn_chunks):
            blk = arr[c, :, :, : sz * free].reshape(nch, P, sz, free)
            # (n, p, s, j) -> rows (m0+s)*P+p, cols n*free+j
            blk = blk.transpose(2, 1, 0, 3).reshape(sz * P, nch * free)
            full[m0 * P : (m0 + sz) * P] = blk
        return full

    out = np.empty((T, D), np.float32)
    for c in range(NCORES):
        osw = np.asarray(res.results[c]["outs"], dtype=np.float32)
        out[c * TS : (c + 1) * TS] = unswz(osw, NCH_S, FREE_S).T
    for e in range(NCORES):
        n_e = int(counts[e])
        if n_e:
            orw = np.asarray(res.results[e]["outr"], dtype=np.float32)
            full = unswz(orw, NCH_R, FREE_R)  # [D, NTOK]
            idx = tok_idx[off[e] : off[e] + n_e]
            out[idx] += full[:, :n_e].T
    return out.reshape(BS, SLEN, D)


# revision 19
# speedup vs baseline: 1.2092x; 1.0024x over previous
"""MoE (token-choice top-2 router + grouped SwiGLU experts + shared expert)
on 8 Trainium2 NeuronCores.

Sharding: expert-parallel — core e owns expert e's routed tokens (host
dispatch, capacity-padded), plus a 1/8 data-parallel slice of the shared
expert. Host does the routing control plane (gate matmul, top-2, stable
sort, gather/scale, final scatter-add combine); the device does all the
FLOPs in bf16 with fp32 PSUM accumulation.

v2 redesign (trace-driven, from the 397us baseline):
- ALL operands are SBUF-resident before use. Weights arrive via
  host-side swizzled DRAM layouts so every DMA moves 11-12KB contiguous
  per-partition lines (the v1 baseline streamed weight tiles as 256B
  descriptors, which capped the weight stream at ~60-80GB/s and starved
  the PE at kernel start and each phase transition).
- Weight tensors are chunked along the output (m) dim into 4 chunks,
  loaded through an 8-slot rotating tile pool: at any time one phase's
  full weight tensor + the next tensor's prefetch are in flight. Slot
  recycling gives the prefetch pipeline for free via tile deps.
- Custom per-phase matmul loops (no composable_matmul_tile_kernel):
  r1 is n-outer so the first x n-chunk + first w1 m-chunk unblock the
  PE ~8us in; r3/out_r are n-inner so consecutive matmuls share the
  stationary weight tile. PSUM pool of 6 banks keeps deep pipelining.
- Outputs staged in SBUF (bf16) and written as 2.9-4KB-line DMAs in a
  swizzled DRAM layout (host unswizzles); final flush is one DMA.

Self-contained: only needs numpy/ml_dtypes/concourse (the Bass stack).
"""

import math
import os

import numpy as np
import ml_dtypes

BF16 = ml_dtypes.bfloat16
NCORES = 8
TOP_K = 2
ROUTE_SCALE = 1.0
P = 128

# filled by the last kernel() call (exec_time_ns etc. when tracing)
LAST = {}

_PROGRAM_CACHE = {}


def _install_profhook():
    """Best-effort shim for antenv.axon_hooks so trace=True can capture NTFF
    profiles in this container. Harmless no-op if anything is missing."""
    try:
        import sys
        import types

        if "antenv.axon_hooks" in sys.modules:
            return
        import trn_agent_boot.trn_boot as tb

        hook = tb._ntff_profile_via_ctypes("/opt/axon/libaxon_pjrt.so")
        m = types.ModuleType("antenv.axon_hooks")
        m._hook = hook
        m.set_axon_ntff_profile_hook = lambda h: setattr(m, "_hook", h)
        m.get_axon_ntff_profile_hook = lambda: m._hook
        import antenv

        sys.modules["antenv.axon_hooks"] = m
        antenv.axon_hooks = m

        import concourse.bass_utils as bu

        bu.upload_artifacts = lambda tmpdir: tmpdir
    except Exception:
        pass


def _free_div(n):
    """Largest f = n/k (k<=4) with f <= 512, preferring big f."""
    for k in (1, 2, 3, 4):
        if n % k == 0 and n // k <= 512:
            return n // k
    for f in (512, 384, 256, 128):
        if n % f == 0:
            return f
    raise ValueError(f"no free-dim divisor for {n}")


def _pick_ntok(nmax, cap):
    """Smallest n in [nmax, cap] whose free-dim divides nicely (PSUM <=512)."""
    for n in range(nmax, cap + 1):
        try:
            _free_div(n)
            return n
        except ValueError:
            continue
    return cap


def _mchunks(n_mtiles, n_chunks):
    """Split n_mtiles 128-col m-tiles into n_chunks contiguous chunks,
    smallest chunk FIRST (the first chunk gates the kernel head)."""
    base = n_mtiles // n_chunks
    rem = n_mtiles % n_chunks
    sizes = [base] * (n_chunks - rem) + [base + 1] * rem
    out = []
    s = 0
    for sz in sizes:
        out.append((s, sz))
        s += sz
    return out


WCH = 4  # m-chunks per weight tensor


def _build_program(D, H, NTOK, TS):
    import concourse.bacc as bacc
    import concourse.tile as tile
    from concourse import mybir
    from contextlib import ExitStack

    bf = mybir.dt.bfloat16
    f32 = mybir.dt.float32

    KD = D // P  # 16 k-subtiles for the D-contraction (up-proj)
    KH = H // P  # 11 k-subtiles for the H-contraction (out-proj)
    MT_H = H // P  # 11 m-tiles over H
    MT_D = D // P  # 16 m-tiles over D
    FREE_R = _free_div(NTOK)
    NCH_R = NTOK // FREE_R
    FREE_S = _free_div(TS)
    NCH_S = TS // FREE_S
    XKC = 2  # k-chunks for the xr prefetch (first-tile latency)
    assert KD % XKC == 0
    KDC = KD // XKC

    up_chunks = _mchunks(MT_H, WCH)  # chunks of H m-tiles (w1/w3/sw1/sw3)
    dn_chunks = _mchunks(MT_D, WCH)  # chunks of D m-tiles (w2/sw2)

    nc = bacc.Bacc(target_bir_lowering=False)

    # --- DRAM tensors (all host-swizzled; per-partition-contiguous lines) ---
    # up-weight chunk c: [P, KD*csz*P] row-major; (p, ks, j) = wT[ks*P+p, c0*P+j]
    def wdecl(name, nk, chunks):
        return [
            nc.dram_tensor(f"{name}{c}", [P, nk * sz * P], bf, kind="ExternalInput")
            for c, (_, sz) in enumerate(chunks)
        ]

    w1d = wdecl("w1", KD, up_chunks)
    w3d = wdecl("w3", KD, up_chunks)
    sw1d = wdecl("sw1", KD, up_chunks)
    sw3d = wdecl("sw3", KD, up_chunks)
    w2d = wdecl("w2", KH, dn_chunks)
    sw2d = wdecl("sw2", KH, dn_chunks)
    # xr chunk (n, kc): [P, KDC*FREE_R]; (p, s, j) = xrT[(kc*KDC+s)*P+p, n*FREE_R+j]
    xrd = [
        [
            nc.dram_tensor(f"xr{n}_{kc}", [P, KDC * FREE_R], bf, kind="ExternalInput")
            for kc in range(XKC)
        ]
        for n in range(NCH_R)
    ]
    xsd = nc.dram_tensor("xs", [P, KD * TS], bf, kind="ExternalInput")
    # outputs (swizzled, host unswizzles): routed [WCH, NCH_R, P, msz*FREE_R]
    outr = nc.dram_tensor(
        "outr", [WCH, NCH_R, P, max(sz for _, sz in dn_chunks) * FREE_R], bf,
        kind="ExternalOutput",
    )
    outs = nc.dram_tensor(
        "outs", [WCH, NCH_S, P, max(sz for _, sz in dn_chunks) * FREE_S], bf,
        kind="ExternalOutput",
    )

    with tile.TileContext(nc) as tc, ExitStack() as ctx:
        caches = ctx.enter_context(tc.tile_pool(name="caches", bufs=1))
        # persistent activation caches
        xr_t = [
            [
                caches.tile(
                    [P, KDC, FREE_R], bf, tag=f"xr{n}_{kc}", name=f"xr{n}_{kc}"
                )
                for kc in range(XKC)
            ]
            for n in range(NCH_R)
        ]
        xs_t = caches.tile([P, KD, TS], bf, tag="xs")
        h1c = caches.tile([P, MT_H, NTOK], bf, tag="h1c")
        h1s = caches.tile([P, MT_H, TS], bf, tag="h1s")

        wpool = ctx.enter_context(tc.tile_pool(name="wpool", bufs=7))
        psum = ctx.enter_context(tc.tile_pool(name="psum", bufs=6, space="PSUM"))
        stgp = ctx.enter_context(tc.tile_pool(name="stg", bufs=1))

        # ---- prefetch issues (engine FIFO order = pacing) ----
        # scalar queue: xr n-chunks then xs
        for n in range(NCH_R):
            for kc in range(XKC):
                nc.scalar.dma_start(
                    out=xr_t[n][kc][:],
                    in_=xrd[n][kc][:].rearrange("p (s j) -> p s j", s=KDC),
                )
        nc.scalar.dma_start(
            out=xs_t[:], in_=xsd[:].rearrange("p (s j) -> p s j", s=KD)
        )

        # sync queue: weight chunks in consumption order; the 8-slot pool
        # rotation makes later tensors' DMAs wait for the earlier tensors'
        # readers automatically (prefetch pipeline).
        def wload(dram_chunks, nk, chunks, label, split0=False):
            # chunk layout [P, mi, ks, 128]: the k-sweep for a fixed m-tile
            # walks contiguous 256B blocks (strided LDWEIGHTS reads measure
            # ~20% slower PE streaming). split0: load chunk 0 with per-m-tile
            # DMAs so the first matmul only gates on one m-tile (subtile deps).
            tiles = []
            for c, (_, sz) in enumerate(chunks):
                t = wpool.tile([P, sz, nk, P], bf, tag="w", name=f"{label}{c}")
                src = dram_chunks[c][:].rearrange("p (m s j) -> p m s j", m=sz, s=nk)
                if split0 and c == 0:
                    for mi in range(sz):
                        nc.sync.dma_start(
                            out=t[:, mi : mi + 1], in_=src[:, mi : mi + 1]
                        )
                else:
                    nc.sync.dma_start(out=t[:], in_=src)
                tiles.append(t)
            return tiles

        w1t = wload(w1d, KD, up_chunks, "w1t", split0=True)
        w3t = wload(w3d, KD, up_chunks, "w3t")
        sw1t = wload(sw1d, KD, up_chunks, "sw1t")
        sw3t = wload(sw3d, KD, up_chunks, "sw3t")
        w2t = wload(w2d, KH, dn_chunks, "w2t")
        sw2t = wload(sw2d, KH, dn_chunks, "sw2t")

        Silu = mybir.ActivationFunctionType.Silu

        def xr_rhs(n, ks):
            return xr_t[n][ks // KDC][:, ks % KDC, :]

        # ---- phase r1: h1 = silu(w1T.T @ xr), n-outer (stream-friendly) ----
        for n in range(NCH_R):
            for c, (m0, msz) in enumerate(up_chunks):
                for mi in range(msz):
                    ps = psum.tile([P, 512], f32, tag="ps", name=f"ps_r1_{n}_{c}_{mi}")
                    for ks in range(KD):
                        nc.tensor.matmul(
                            ps[:, :FREE_R],
                            w1t[c][:, mi, ks, :],
                            xr_rhs(n, ks),
                            start=(ks == 0),
                            stop=(ks == KD - 1),
                        )
                    nc.scalar.activation(
                        h1c[:, m0 + mi, n * FREE_R : (n + 1) * FREE_R],
                        ps[:, :FREE_R],
                        Silu,
                    )

        # ---- phase r3: h1 *= (w3T.T @ xr), n-inner (weight reuse) ----
        for c, (m0, msz) in enumerate(up_chunks):
            for mi in range(msz):
                pss = [
                    psum.tile([P, 512], f32, tag="ps", name=f"ps_r3_{c}_{mi}_{n}")
                    for n in range(NCH_R)
                ]
                for ks in range(KD):
                    for n in range(NCH_R):
                        nc.tensor.matmul(
                            pss[n][:, :FREE_R],
                            w3t[c][:, mi, ks, :],
                            xr_rhs(n, ks),
                            start=(ks == 0),
                            stop=(ks == KD - 1),
                        )
                for n in range(NCH_R):
                    sl = h1c[:, m0 + mi, n * FREE_R : (n + 1) * FREE_R]
                    nc.vector.tensor_mul(out=sl, in0=pss[n][:, :FREE_R], in1=sl)

        # ---- phase s1/s3: shared-expert swiglu on xs ----
        for wt, is_mul in ((sw1t, False), (sw3t, True)):
            for c, (m0, msz) in enumerate(up_chunks):
                for mi in range(msz):
                    pss = [
                        psum.tile([P, 512], f32, tag="ps", name=f"ps_s_{c}_{mi}_{n}")
                        for n in range(NCH_S)
                    ]
                    for ks in range(KD):
                        for n in range(NCH_S):
                            nc.tensor.matmul(
                                pss[n][:, :FREE_S],
                                wt[c][:, mi, ks, :],
                                xs_t[:, ks, n * FREE_S : (n + 1) * FREE_S],
                                start=(ks == 0),
                                stop=(ks == KD - 1),
                            )
                    for n in range(NCH_S):
                        sl = h1s[:, m0 + mi, n * FREE_S : (n + 1) * FREE_S]
                        if is_mul:
                            nc.vector.tensor_mul(
                                out=sl, in0=pss[n][:, :FREE_S], in1=sl
                            )
                        else:
                            nc.scalar.activation(sl, pss[n][:, :FREE_S], Silu)

        # ---- phase out_r: outrT = w2T.T @ h1 (n-inner; vector copies,
        # scalar DMAs) ----
        for c, (m0, msz) in enumerate(dn_chunks):
            stgs = [
                stgp.tile([P, msz, FREE_R], bf, tag="stgr", bufs=4, name=f"stgr{c}_{n}")
                for n in range(NCH_R)
            ]
            for mi in range(msz):
                pss = [
                    psum.tile([P, 512], f32, tag="ps", name=f"ps_or_{c}_{mi}_{n}")
                    for n in range(NCH_R)
                ]
                for ks in range(KH):
                    for n in range(NCH_R):
                        nc.tensor.matmul(
                            pss[n][:, :FREE_R],
                            w2t[c][:, mi, ks, :],
                            h1c[:, ks, n * FREE_R : (n + 1) * FREE_R],
                            start=(ks == 0),
                            stop=(ks == KH - 1),
                        )
                for n in range(NCH_R):
                    nc.vector.tensor_copy(
                        out=stgs[n][:, mi, :], in_=pss[n][:, :FREE_R]
                    )
            for n in range(NCH_R):
                nc.scalar.dma_start(
                    out=outr[c, n, :, : msz * FREE_R].rearrange(
                        "p (s j) -> p s j", s=msz
                    ),
                    in_=stgs[n][:],
                )

        # ---- phase out_s: outsT = sw2T.T @ h1s (scalar copies, sync DMAs) ----
        for c, (m0, msz) in enumerate(dn_chunks):
            stgs = [
                stgp.tile([P, msz, FREE_S], bf, tag="stgs", bufs=2, name=f"stgs{c}_{n}")
                for n in range(NCH_S)
            ]
            for mi in range(msz):
                pss = [
                    psum.tile([P, 512], f32, tag="ps", name=f"ps_os_{c}_{mi}_{n}")
                    for n in range(NCH_S)
                ]
                for ks in range(KH):
                    for n in range(NCH_S):
                        nc.tensor.matmul(
                            pss[n][:, :FREE_S],
                            sw2t[c][:, mi, ks, :],
                            h1s[:, ks, n * FREE_S : (n + 1) * FREE_S],
                            start=(ks == 0),
                            stop=(ks == KH - 1),
                        )
                for n in range(NCH_S):
                    nc.scalar.activation(
                        stgs[n][:, mi, :],
                        pss[n][:, :FREE_S],
                        mybir.ActivationFunctionType.Copy,
                    )
                # per-mi output DMA: the final flush after the last matmul
                # is one m-tile, not a whole chunk (shrinks the kernel tail)
                for n in range(NCH_S):
                    nc.sync.dma_start(
                        out=outs[c, n, :, : msz * FREE_S].rearrange(
                            "p (s j) -> p s j", s=msz
                        )[:, mi : mi + 1],
                        in_=stgs[n][:, mi : mi + 1],
                    )

    nc.compile()
    return nc


def _route(x, gate_w, expert_bias):
    """Host control plane mirroring the reference routing exactly."""
    BS, SLEN, D = x.shape
    T = BS * SLEN
    xt = np.ascontiguousarray(x.reshape(T, D), dtype=np.float32)
    logits = xt @ gate_w.astype(np.float32).T  # [T, E]
    scores = 1.0 / (1.0 + np.exp(-logits))
    biased = scores + np.asarray(expert_bias, np.float32)[None, :]
    sel = np.argsort(-biased, axis=1, kind="stable")[:, :TOP_K]  # [T, K]
    top_scores = np.take_along_axis(scores, sel, axis=1) * ROUTE_SCALE
    sel_flat = sel.reshape(-1)
    order = np.argsort(sel_flat, kind="stable")  # [T*K]
    counts = np.bincount(sel_flat, minlength=NCORES)
    tok_idx = order // TOP_K
    scores_sorted = top_scores.reshape(-1)[order].astype(np.float32)
    return xt, counts, tok_idx, scores_sorted


def _swz_w(wT, nk, chunks):
    """wT [K, M] f32 -> list of [P, sz*nk*P] bf16 swizzled chunks with
    per-partition layout (mi, ks, j): the device k-sweep for a fixed m-tile
    reads contiguous 256B blocks."""
    K, M = wT.shape
    w3d = wT.reshape(nk, P, M)  # (ks, p, m)
    out = []
    for m0, sz in chunks:
        blk = w3d[:, :, m0 * P : (m0 + sz) * P].reshape(nk, P, sz, P)
        # (ks, p, mi, j) -> (p, mi, ks, j)
        out.append(
            np.ascontiguousarray(blk.transpose(1, 2, 0, 3))
            .reshape(P, sz * nk * P)
            .astype(BF16)
        )
    return out


def kernel(x, gate_w, w1, w2, w3, sw1, sw2, sw3, expert_bias):
    from concourse.bass_utils import run_bass_kernel_spmd

    x = np.asarray(x, np.float32)
    gate_w = np.asarray(gate_w, np.float32)
    w1 = np.asarray(w1, np.float32)
    w2 = np.asarray(w2, np.float32)
    w3 = np.asarray(w3, np.float32)
    sw1 = np.asarray(sw1, np.float32)
    sw2 = np.asarray(sw2, np.float32)
    sw3 = np.asarray(sw3, np.float32)
    expert_bias = np.asarray(expert_bias, np.float32)
    BS, SLEN, D = x.shape
    T = BS * SLEN
    H = w1.shape[1]
    TS = T // NCORES
    KD = D // P
    KH = H // P
    MT_H = H // P
    MT_D = D // P

    xt, counts, tok_idx, scores_sorted = _route(x, gate_w, expert_bias)
    off = np.concatenate([[0], np.cumsum(counts)]).astype(np.int64)
    CAP = max(128, int(math.ceil(counts.max() / 128) * 128))
    NTOK = _pick_ntok(max(128, int(counts.max())), CAP)
    FREE_R = _free_div(NTOK)
    NCH_R = NTOK // FREE_R
    FREE_S = _free_div(TS)
    NCH_S = TS // FREE_S
    XKC = 2
    KDC = KD // XKC
    up_chunks = _mchunks(MT_H, WCH)
    dn_chunks = _mchunks(MT_D, WCH)

    key = (D, H, NTOK, TS)
    if key not in _PROGRAM_CACHE:
        _PROGRAM_CACHE[key] = _build_program(D, H, NTOK, TS)
    nc = _PROGRAM_CACHE[key]

    # ---- stage per-core inputs (swizzled) ----
    shared_chunks = {}
    for name, wmat, nk, chunks in (
        ("sw1", sw1.T, KD, up_chunks),
        ("sw3", sw3.T, KD, up_chunks),
        ("sw2", sw2.T, KH, dn_chunks),
    ):
        for c, arr in enumerate(_swz_w(np.ascontiguousarray(wmat), nk, chunks)):
            shared_chunks[f"{name}{c}"] = arr

    in_maps = []
    for e in range(NCORES):
        n_e = int(counts[e])
        idx = tok_idx[off[e] : off[e] + n_e]
        seg = xt[idx] * scores_sorted[off[e] : off[e] + n_e, None]  # [n_e, D]
        xrT = np.zeros((D, NTOK), np.float32)
        xrT[:, :n_e] = seg.T
        xr4 = xrT.reshape(XKC, KDC, P, NTOK)  # (kc, s, p, tok)
        im = dict(shared_chunks)
        for n in range(NCH_R):
            for kc in range(XKC):
                blk = xr4[kc, :, :, n * FREE_R : (n + 1) * FREE_R]
                im[f"xr{n}_{kc}"] = (
                    np.ascontiguousarray(blk.transpose(1, 0, 2))
                    .reshape(P, KDC * FREE_R)
                    .astype(BF16)
                )
        xsT = np.ascontiguousarray(xt[e * TS : (e + 1) * TS].T)  # [D, TS]
        im["xs"] = (
            xsT.reshape(KD, P, TS).transpose(1, 0, 2).reshape(P, KD * TS).astype(BF16)
        )
        for name, wmat, nk, chunks in (
            ("w1", w1[e].T, KD, up_chunks),
            ("w3", w3[e].T, KD, up_chunks),
            ("w2", w2[e].T, KH, dn_chunks),
        ):
            for c, arr in enumerate(
                _swz_w(np.ascontiguousarray(wmat), nk, chunks)
            ):
                im[f"{name}{c}"] = arr
        in_maps.append(im)

    trace = os.environ.get("KERNEL_TRACE", "") not in ("", "0")
    if trace:
        _install_profhook()
    res = run_bass_kernel_spmd(nc, in_maps, list(range(NCORES)), trace=trace)
    LAST["exec_time_ns"] = res.exec_time_ns
    LAST["results"] = res

    # ---- combine: unswizzle outputs, shared slices + routed scatter-add ----
    mszs = [sz for _, sz in dn_chunks]
    msz_max = max(mszs)

    def unswz(arr, nch, free):
        # arr [WCH, nch, P, msz_max*free] -> [D, nch*free]
        full = np.empty((MT_D * P, nch * free), np.float32)
        for c, (m0, sz) in enumerate(dn_chunks):
            blk = arr[c, :, :, : sz * free].reshape(nch, P, sz, free)
            # (n, p, s, j) -> rows (m0+s)*P+p, cols n*free+j
            blk = blk.transpose(2, 1, 0, 3).reshape(sz * P, nch * free)
            full[m0 * P : (m0 + sz) * P] = blk
        return full

    out = np.empty((T, D), np.float32)
    for c in range(NCORES):
        osw = np.asarray(res.results[c]["outs"], dtype=np.float32)
        out[c * TS : (c + 1) * TS] = unswz(osw, NCH_S, FREE_S).T
    for e in range(NCORES):
        n_e = int(counts[e])
        if n_e:
            orw = np.asarray(res.results[e]["outr"], dtype=np.float32)
            full = unswz(orw, NCH_R, FREE_R)  # [D, NTOK]
            idx = tok_idx[off[e] : off[e] + n_e]
            out[idx] += full[:, :n_e].T
    return out.reshape(BS, SLEN, D)
